# revision 4
# baseline (speedup 1.0000x reference)
"""GroupGAT kernel for Trainium2 (Bass/Tile), 8-core data-parallel.

Math restructure (attention weights commute with @W):
    e[b,n] = lrelu(h_self.(W a1) + h[b,n].(W a2))    <- dots in h-space
    out = elu((h_self + hw_ally) @ W_ally + hw_opp @ W_opp),
    hw_x[b,:] = sum_n w_x[b,n] h[b,n,:]              <- weighted sums in h-space

v5 design (per 128-row tile):
  - NO on-chip transpose. The host supplies BOTH layouts of h from HBM:
    h_t [b, n*d] (b-part, wsum stationaries) and hT [d, n, b] (d-part,
    dot stationaries). 2x HBM reads beat the xbar transpose (256B
    descriptors at ~13.6GB/s/queue vs ~21GB/s/queue HBM) and free ACT.
  - dots: 41 PE matmuls lhsT=hT[:,n,:] x rhs=vcat[d,4] -> e_ps[b,n,4].
  - diag[p,j,d] = w01[p,j]*(d==p): GPSIMD affine_select is the cheapest
    materializer (135ns/slot vs DVE 233 / ACT 390) -> slots 8..41 on
    GPSIMD, 0..7 on DVE.
  - wsums: 42 PE matmuls lhsT=h_j (b-part) x rhs=diag_j -> hwT[d,b]
    accumulated in PSUM (diag trick scales + transposes in one pass).
  - mneg preloaded for all tiles in one DMA; outputs accumulate in a
    persistent SBUF buffer, stored with one DMA at the end (kills ~4k
    small descriptors).
  - 6-stage pipeline, one tile per stage per iteration; per-engine
    instruction streams ordered oldest-dependency-first so no engine
    head-of-line blocks:
      load(k) | dots(k-2) | e+softmax(k-3) | diag(k-4) | wsum+xT(k-5)
      | finals+elu(k-6)
"""

import numpy as np
import ml_dtypes

import concourse.bass as bass
import concourse.bacc as bacc
import concourse.mybir as mybir
from concourse import tile
from concourse.bass_utils import run_bass_kernel_spmd

N_CORES = 8
B = 16384
NN = 41
NA = 20
NO = 20
D = 128
B_SHARD = B // N_CORES
P = 128
N_TILES = B_SHARD // P
NEG_INF = -1e9
NJ = 42  # diag slots: 0..20 ally (h nodes 0..20), 21..41 opp (h nodes 0,21..40)

F32 = mybir.dt.float32
BF16 = mybir.dt.bfloat16
AL = mybir.AluOpType
AF = mybir.ActivationFunctionType
BF16_NP = ml_dtypes.bfloat16

DVE_NSLOT = 8  # diag slots 0..7 on DVE, 8..41 on GPSIMD


def _h_node_of_slot(j):
    if j <= 20:
        return j
    if j == 21:
        return 0
    return j - 1  # 22..41 -> h nodes 21..40


def build_nc(b_shard=B_SHARD):
    n_tiles = b_shard // P
    nc = bacc.Bacc("TRN2", target_bir_lowering=False, debug=False)

    h_d = nc.dram_tensor("h", [b_shard, NN * D], BF16, kind="ExternalInput").ap()
    hT_d = nc.dram_tensor("hT", [b_shard, NN * D], BF16, kind="ExternalInput").ap()
    # mneg pre-shuffled on host to [b_in_tile, tile, col]
    mneg_d = nc.dram_tensor("mneg", [P, n_tiles * NJ], F32, kind="ExternalInput").ap()
    vcat_d = nc.dram_tensor("vcat", [D, 4], BF16, kind="ExternalInput").ap()
    wcat_d = nc.dram_tensor("wcat", [D, 2 * D], BF16, kind="ExternalInput").ap()
    maskf_d = nc.dram_tensor("maskf", [P, NJ * D], BF16, kind="ExternalInput").ap()
    # out stored as [b_in_tile, tile, col]; host unshuffles
    out_d = nc.dram_tensor("out", [P, n_tiles * D], F32, kind="ExternalOutput").ap()

    with tile.TileContext(nc) as tc:
        with (
            tc.tile_pool(name="const", bufs=1) as cpool,
            tc.tile_pool(name="hin", bufs=7) as hpool,
            tc.tile_pool(name="htin", bufs=4) as htpool,
            tc.tile_pool(name="diag", bufs=3) as dpool,
            tc.tile_pool(name="small", bufs=5) as spool,
            tc.tile_pool(name="work", bufs=3) as wpool,
            tc.tile_pool(name="psum_e", bufs=2, space=bass.MemorySpace.PSUM) as ppool_e,
            tc.tile_pool(name="psum_hw", bufs=2, space=bass.MemorySpace.PSUM) as ppool_hw,
            tc.tile_pool(name="psum_o", bufs=2, space=bass.MemorySpace.PSUM) as ppool_o,
        ):
            vcat = cpool.tile([D, 4], BF16)
            wcat = cpool.tile([D, 2 * D], BF16)
            maskf = cpool.tile([P, NJ, D], BF16)
            mneg_sb = cpool.tile([P, n_tiles, NJ], F32)
            out_sb = cpool.tile([P, n_tiles, D], F32)
            nc.sync.dma_start(vcat[:], vcat_d[:])
            nc.sync.dma_start(wcat[:], wcat_d[:])
            nc.sync.dma_start(maskf[:], maskf_d[:])
            nc.sync.dma_start(mneg_sb[:], mneg_d[:])

            st_load = {}
            st_eps = {}
            st_w01 = {}
            st_diag = {}
            st_hw = {}
            st_ops = {}

            def phase_load(it):
                b0 = it * P
                h_t = hpool.tile([P, NN * D], BF16)
                hT = htpool.tile([P, NN, D], BF16)  # hT[d, n, b]
                nc.sync.dma_start(h_t[:], h_d[b0 : b0 + P])
                nc.sync.dma_start(hT[:], hT_d[b0 : b0 + P])
                st_load[it] = (h_t, hT)

            def phase_dots(it):
                _, hT = st_load[it]
                e_ps = ppool_e.tile([P, NN, 4], F32, tag="eps")
                for n in range(NN):
                    nc.tensor.matmul(
                        e_ps[:, n, :], hT[:, n, :], vcat[:], start=True, stop=True
                    )
                st_eps[it] = e_ps

            def phase_e(it):
                e_ps = st_eps.pop(it)
                e_pre = spool.tile([P, NJ], F32, tag="epre")
                s1a = e_ps[:, 0:1, 0]
                s1o = e_ps[:, 0:1, 2]
                nc.vector.scalar_tensor_tensor(
                    e_pre[:, 0:21], e_ps[:, 0:21, 1], s1a,
                    mneg_sb[:, it, 0:21], AL.add, AL.add,
                )
                nc.vector.tensor_scalar_add(e_pre[:, 21:22], e_ps[:, 0:1, 3], s1o)
                nc.vector.scalar_tensor_tensor(
                    e_pre[:, 22:42], e_ps[:, 21:NN, 3], s1o,
                    mneg_sb[:, it, 22:42], AL.add, AL.add,
                )
                nc.vector.scalar_tensor_tensor(
                    e_pre[:], e_pre[:], 0.2, e_pre[:], AL.mult, AL.max
                )
                expe = spool.tile([P, NJ], F32, tag="expe")
                den = spool.tile([P, 2], F32, tag="den")
                rec = spool.tile([P, 2], F32, tag="rec")
                nc.scalar.activation(
                    expe[:, 0:21], e_pre[:, 0:21], AF.Exp, accum_out=den[:, 0:1]
                )
                nc.scalar.activation(
                    expe[:, 21:42], e_pre[:, 21:42], AF.Exp, accum_out=den[:, 1:2]
                )
                nc.vector.reciprocal(rec[:], den[:])
                w01 = spool.tile([P, NJ], BF16, tag="w01")
                nc.vector.tensor_scalar_mul(w01[:, 0:21], expe[:, 0:21], rec[:, 0:1])
                nc.vector.tensor_scalar_mul(w01[:, 21:42], expe[:, 21:42], rec[:, 1:2])
                nc.vector.tensor_scalar_add(w01[:, 0:1], w01[:, 0:1], 1.0)
                st_w01[it] = w01

            def phase_diag(it):
                w01 = st_w01.pop(it)
                diag = dpool.tile([P, NJ, D], BF16)
                nc.vector.tensor_mul(
                    diag[:, 0:DVE_NSLOT, :],
                    maskf[:, 0:DVE_NSLOT, :],
                    w01[:, 0:DVE_NSLOT, None].broadcast_to([P, DVE_NSLOT, D]),
                )
                nc.gpsimd.affine_select(
                    diag[:, DVE_NSLOT:NJ, :],
                    w01[:, DVE_NSLOT:NJ, None].broadcast_to([P, NJ - DVE_NSLOT, D]),
                    pattern=[[0, NJ - DVE_NSLOT], [1, D]],
                    compare_op=AL.is_equal,
                    fill=0.0,
                    base=0,
                    channel_multiplier=-1,
                )
                st_diag[it] = diag

            def phase_wsum(it):
                h_t, _ = st_load.pop(it)
                diag = st_diag.pop(it)

                def hnode(n):
                    return h_t[:, n * D : (n + 1) * D]

                # hwT[d, b] += h_n[b,d]*w01[b,j]  (diag trick)
                hwps = ppool_hw.tile([P, 2, D], F32, tag="hw")
                for grp in (0, 1):
                    hwT = hwps[:, grp, :]
                    for k in range(21):
                        j = grp * 21 + k
                        nc.tensor.matmul(
                            hwT, hnode(_h_node_of_slot(j)), diag[:, j, :],
                            start=(k == 0), stop=(k == 20),
                        )
                st_hw[it] = hwps

            def phase_copies(it):
                hwps = st_hw.pop(it)
                xT_a = wpool.tile([P, D], BF16, tag="xta")
                xT_o = wpool.tile([P, D], BF16, tag="xto")
                nc.scalar.copy(xT_a[:], hwps[:, 0, :])
                nc.scalar.copy(xT_o[:], hwps[:, 1, :])
                st_ops[it] = (xT_a, xT_o)

            def phase_fin_pe(it):
                xT_a, xT_o = st_ops[it]
                out_ps = ppool_o.tile([P, D], F32, tag="ops")
                nc.tensor.matmul(out_ps[:], xT_a[:], wcat[:, 0:D], start=True, stop=False)
                nc.tensor.matmul(out_ps[:], xT_o[:], wcat[:, D : 2 * D], start=False, stop=True)
                st_ops[it] = out_ps

            def phase_elu(it):
                out_ps = st_ops.pop(it)
                # elu(x) = max(x, exp(min(x,0)) - 1)
                t1 = wpool.tile([P, D], F32, tag="t1")
                nc.vector.tensor_scalar_min(t1[:], out_ps[:], 0.0)
                nc.scalar.activation(t1[:], t1[:], AF.Exp)
                nc.vector.scalar_tensor_tensor(
                    out_sb[:, it, :], t1[:], -1.0, out_ps[:], AL.add, AL.max
                )

            # 6-stage pipeline; emission order makes each engine's stream
            # oldest-dependency-first.
            for k in range(n_tiles + 6):
                if k < n_tiles:
                    phase_load(k)
                if 6 <= k:
                    phase_fin_pe(k - 6)
                if 4 <= k < n_tiles + 4:
                    phase_diag(k - 4)
                if 2 <= k < n_tiles + 2:
                    phase_dots(k - 2)
                if 3 <= k < n_tiles + 3:
                    phase_e(k - 3)
                if 5 <= k < n_tiles + 5:
                    phase_wsum(k - 5)
                    phase_copies(k - 5)
                if 6 <= k:
                    phase_elu(k - 6)

            nc.sync.dma_start(out_d[:], out_sb[:])

    nc.compile()
    return nc


_NC_CACHE = {}


def _get_nc(b_shard):
    if b_shard not in _NC_CACHE:
        _NC_CACHE[b_shard] = build_nc(b_shard)
    return _NC_CACHE[b_shard]


def _host_precompute(W_ally, W_opp, a_ally, a_opp, mask):
    v1a = W_ally @ a_ally[:D, 0]
    v2a = W_ally @ a_ally[D:, 0]
    v1o = W_opp @ a_opp[:D, 0]
    v2o = W_opp @ a_opp[D:, 0]
    vcat = np.ascontiguousarray(np.stack([v1a, v2a, v1o, v2o], axis=1).astype(BF16_NP))
    wcat = np.ascontiguousarray(np.concatenate([W_ally, W_opp], axis=1).astype(BF16_NP))
    eye = (np.arange(P)[:, None] == np.arange(D)[None, :]).astype(BF16_NP)
    maskf = np.ascontiguousarray(
        np.repeat(eye[:, None, :], NJ, axis=1).reshape(P, NJ * D)
    )
    b = mask.shape[0]
    mneg = np.zeros((b, NJ), np.float32)
    mneg[:, 1:21] = np.where(mask[:, 1 : 1 + NA], NEG_INF, 0.0)
    mneg[:, 22:42] = np.where(mask[:, 1 + NA :], NEG_INF, 0.0)
    return vcat, wcat, maskf, mneg


def kernel(h, W_ally, W_opp, a_ally, a_opp, mask, num_ally, num_opp):
    assert int(num_ally) == NA and int(num_opp) == NO
    h = np.asarray(h, dtype=np.float32)
    mask = np.asarray(mask)
    W_ally = np.asarray(W_ally, dtype=np.float32)
    W_opp = np.asarray(W_opp, dtype=np.float32)
    a_ally = np.asarray(a_ally, dtype=np.float32)
    a_opp = np.asarray(a_opp, dtype=np.float32)

    vcat, wcat, maskf, mneg = _host_precompute(W_ally, W_opp, a_ally, a_opp, mask)
    bfull = h.shape[0]
    h_bf3 = h.astype(BF16_NP)  # [B, NN, D]
    h_bf = np.ascontiguousarray(h_bf3.reshape(bfull, NN * D))
    # hT layout: per tile t of 128 rows, hT[t*128 + d, n*128 + b] = h[t*128 + b, n, d]
    n_tiles_full = bfull // P
    hT_bf = np.ascontiguousarray(
        h_bf3.reshape(n_tiles_full, P, NN, D).transpose(0, 3, 2, 1)
    ).reshape(bfull, NN * D)
    # mneg shuffled to [b_in_tile, tile, col] per core
    mneg_sh = np.ascontiguousarray(
        mneg.reshape(N_CORES, N_TILES, P, NJ).transpose(0, 2, 1, 3)
    ).reshape(N_CORES, P, N_TILES * NJ)

    nc = _get_nc(B_SHARD)
    in_maps = []
    for c in range(N_CORES):
        s = slice(c * B_SHARD, (c + 1) * B_SHARD)
        in_maps.append(
            {
                "h": h_bf[s],
                "hT": hT_bf[s],
                "mneg": mneg_sh[c],
                "vcat": vcat,
                "wcat": wcat,
                "maskf": maskf,
            }
        )
    res = run_bass_kernel_spmd(nc, in_maps, core_ids=list(range(N_CORES)))
    global LAST_RESULTS
    LAST_RESULTS = res
    # out stored [b_in_tile, tile, col] -> [tile*128 + b, col]
    return np.concatenate(
        [
            np.ascontiguousarray(
                res.results[c]["out"].reshape(P, N_TILES, D).transpose(1, 0, 2)
            ).reshape(B_SHARD, D)
            for c in range(N_CORES)
        ],
        axis=0,
    )


LAST_RESULTS = None


# revision 6
# speedup vs baseline: 1.4023x; 1.4023x over previous
"""GroupGAT kernel for Trainium2 (Bass/Tile), 8-core data-parallel.

Math restructure (attention weights commute with @W):
    e[b,n] = lrelu(h_self.(W a1) + h[b,n].(W a2))    <- dots in h-space
    out = elu((h_self + hw_ally) @ W_ally + hw_opp @ W_opp),
    hw_x[b,:] = sum_n w_x[b,n] h[b,n,:]              <- weighted sums in h-space

v5 design (per 128-row tile):
  - NO on-chip transpose. The host supplies BOTH layouts of h from HBM:
    h_t [b, n*d] (b-part, wsum stationaries) and hT [d, n, b] (d-part,
    dot stationaries). 2x HBM reads beat the xbar transpose (256B
    descriptors at ~13.6GB/s/queue vs ~21GB/s/queue HBM) and free ACT.
  - dots: 41 PE matmuls lhsT=hT[:,n,:] x rhs=vcat[d,4] -> e_ps[b,n,4].
  - diag[p,j,d] = w01[p,j]*(d==p): GPSIMD affine_select is the cheapest
    materializer (135ns/slot vs DVE 233+ / ACT 390) and generates the
    eye pattern itself -> ALL 42 slots in one GPSIMD op, no maskf
    constant, DVE keeps only the e-chain + elu.
  - wsums: 42 PE matmuls lhsT=h_j (b-part) x rhs=diag_j -> hwT[d,b]
    accumulated in PSUM (diag trick scales + transposes in one pass).
  - mneg preloaded for all tiles in one DMA; outputs accumulate in a
    persistent SBUF buffer, stored with one DMA at the end (kills ~4k
    small descriptors).
  - 6-stage pipeline, one tile per stage per iteration; per-engine
    instruction streams ordered oldest-dependency-first so no engine
    head-of-line blocks:
      load(k) | dots(k-2) | e+softmax(k-3) | diag(k-4) | wsum+xT(k-5)
      | finals+elu(k-6)
"""

import numpy as np
import ml_dtypes

import concourse.bass as bass
import concourse.bacc as bacc
import concourse.mybir as mybir
from concourse import tile
from concourse.bass_utils import run_bass_kernel_spmd

N_CORES = 8
B = 16384
NN = 41
NA = 20
NO = 20
D = 128
B_SHARD = B // N_CORES
P = 128
N_TILES = B_SHARD // P
NEG_INF = -1e9
NJ = 42  # diag slots: 0..20 ally (h nodes 0..20), 21..41 opp (h nodes 0,21..40)

F32 = mybir.dt.float32
BF16 = mybir.dt.bfloat16
AL = mybir.AluOpType
AF = mybir.ActivationFunctionType
BF16_NP = ml_dtypes.bfloat16

def _h_node_of_slot(j):
    if j <= 20:
        return j
    if j == 21:
        return 0
    return j - 1  # 22..41 -> h nodes 21..40


def build_nc(b_shard=B_SHARD):
    n_tiles = b_shard // P
    nc = bacc.Bacc("TRN2", target_bir_lowering=False, debug=False)

    h_d = nc.dram_tensor("h", [b_shard, NN * D], BF16, kind="ExternalInput").ap()
    hT_d = nc.dram_tensor("hT", [b_shard, NN * D], BF16, kind="ExternalInput").ap()
    # mneg pre-shuffled on host to [b_in_tile, tile, col]
    mneg_d = nc.dram_tensor("mneg", [P, n_tiles * NJ], F32, kind="ExternalInput").ap()
    vcat_d = nc.dram_tensor("vcat", [D, 4], BF16, kind="ExternalInput").ap()
    wcat_d = nc.dram_tensor("wcat", [D, 2 * D], BF16, kind="ExternalInput").ap()
    # out stored as [b_in_tile, tile, col]; host unshuffles
    out_d = nc.dram_tensor("out", [P, n_tiles * D], F32, kind="ExternalOutput").ap()

    with tile.TileContext(nc) as tc:
        with (
            tc.tile_pool(name="const", bufs=1) as cpool,
            tc.tile_pool(name="hin", bufs=7) as hpool,
            tc.tile_pool(name="htin", bufs=4) as htpool,
            tc.tile_pool(name="diag", bufs=3) as dpool,
            tc.tile_pool(name="small", bufs=5) as spool,
            tc.tile_pool(name="work", bufs=3) as wpool,
            tc.tile_pool(name="psum_e", bufs=2, space=bass.MemorySpace.PSUM) as ppool_e,
            tc.tile_pool(name="psum_hw", bufs=2, space=bass.MemorySpace.PSUM) as ppool_hw,
            tc.tile_pool(name="psum_o", bufs=2, space=bass.MemorySpace.PSUM) as ppool_o,
        ):
            vcat = cpool.tile([D, 4], BF16)
            wcat = cpool.tile([D, 2 * D], BF16)
            mneg_sb = cpool.tile([P, n_tiles, NJ], F32)
            out_sb = cpool.tile([P, n_tiles, D], F32)
            nc.sync.dma_start(vcat[:], vcat_d[:])
            nc.sync.dma_start(wcat[:], wcat_d[:])
            nc.sync.dma_start(mneg_sb[:], mneg_d[:])

            st_load = {}
            st_eps = {}
            st_w01 = {}
            st_diag = {}
            st_hw = {}
            st_ops = {}

            def phase_load(it):
                b0 = it * P
                h_t = hpool.tile([P, NN * D], BF16)
                hT = htpool.tile([P, NN, D], BF16)  # hT[d, n, b]
                nc.sync.dma_start(h_t[:], h_d[b0 : b0 + P])
                nc.sync.dma_start(hT[:], hT_d[b0 : b0 + P])
                st_load[it] = (h_t, hT)

            def phase_dots(it):
                _, hT = st_load[it]
                e_ps = ppool_e.tile([P, NN, 4], F32, tag="eps")
                for n in range(NN):
                    nc.tensor.matmul(
                        e_ps[:, n, :], hT[:, n, :], vcat[:], start=True, stop=True
                    )
                st_eps[it] = e_ps

            def phase_e(it):
                e_ps = st_eps.pop(it)
                e_pre = spool.tile([P, NJ], F32, tag="epre")
                s1a = e_ps[:, 0:1, 0]
                s1o = e_ps[:, 0:1, 2]
                nc.vector.scalar_tensor_tensor(
                    e_pre[:, 0:21], e_ps[:, 0:21, 1], s1a,
                    mneg_sb[:, it, 0:21], AL.add, AL.add,
                )
                nc.vector.tensor_scalar_add(e_pre[:, 21:22], e_ps[:, 0:1, 3], s1o)
                nc.vector.scalar_tensor_tensor(
                    e_pre[:, 22:42], e_ps[:, 21:NN, 3], s1o,
                    mneg_sb[:, it, 22:42], AL.add, AL.add,
                )
                nc.vector.scalar_tensor_tensor(
                    e_pre[:], e_pre[:], 0.2, e_pre[:], AL.mult, AL.max
                )
                expe = spool.tile([P, NJ], F32, tag="expe")
                den = spool.tile([P, 2], F32, tag="den")
                rec = spool.tile([P, 2], F32, tag="rec")
                nc.scalar.activation(
                    expe[:, 0:21], e_pre[:, 0:21], AF.Exp, accum_out=den[:, 0:1]
                )
                nc.scalar.activation(
                    expe[:, 21:42], e_pre[:, 21:42], AF.Exp, accum_out=den[:, 1:2]
                )
                nc.vector.reciprocal(rec[:], den[:])
                w01 = spool.tile([P, NJ], BF16, tag="w01")
                nc.vector.tensor_scalar_mul(w01[:, 0:21], expe[:, 0:21], rec[:, 0:1])
                nc.vector.tensor_scalar_mul(w01[:, 21:42], expe[:, 21:42], rec[:, 1:2])
                nc.vector.tensor_scalar_add(w01[:, 0:1], w01[:, 0:1], 1.0)
                st_w01[it] = w01

            def phase_diag(it):
                w01 = st_w01.pop(it)
                diag = dpool.tile([P, NJ, D], BF16)
                nc.gpsimd.affine_select(
                    diag[:],
                    w01[:, :, None].broadcast_to([P, NJ, D]),
                    pattern=[[0, NJ], [1, D]],
                    compare_op=AL.is_equal,
                    fill=0.0,
                    base=0,
                    channel_multiplier=-1,
                )
                st_diag[it] = diag

            def phase_wsum(it):
                h_t, _ = st_load.pop(it)
                diag = st_diag.pop(it)

                def hnode(n):
                    return h_t[:, n * D : (n + 1) * D]

                # hwT[d, b] += h_n[b,d]*w01[b,j]  (diag trick)
                hwps = ppool_hw.tile([P, 2, D], F32, tag="hw")
                for grp in (0, 1):
                    hwT = hwps[:, grp, :]
                    for k in range(21):
                        j = grp * 21 + k
                        nc.tensor.matmul(
                            hwT, hnode(_h_node_of_slot(j)), diag[:, j, :],
                            start=(k == 0), stop=(k == 20),
                        )
                st_hw[it] = hwps

            def phase_copies(it):
                hwps = st_hw.pop(it)
                xT_a = wpool.tile([P, D], BF16, tag="xta")
                xT_o = wpool.tile([P, D], BF16, tag="xto")
                nc.scalar.copy(xT_a[:], hwps[:, 0, :])
                nc.scalar.copy(xT_o[:], hwps[:, 1, :])
                st_ops[it] = (xT_a, xT_o)

            def phase_fin_pe(it):
                xT_a, xT_o = st_ops[it]
                out_ps = ppool_o.tile([P, D], F32, tag="ops")
                nc.tensor.matmul(out_ps[:], xT_a[:], wcat[:, 0:D], start=True, stop=False)
                nc.tensor.matmul(out_ps[:], xT_o[:], wcat[:, D : 2 * D], start=False, stop=True)
                st_ops[it] = out_ps

            def phase_elu(it):
                out_ps = st_ops.pop(it)
                # elu(x) = max(x, exp(min(x,0)) - 1)
                t1 = wpool.tile([P, D], F32, tag="t1")
                nc.vector.tensor_scalar_min(t1[:], out_ps[:], 0.0)
                nc.scalar.activation(t1[:], t1[:], AF.Exp)
                nc.vector.scalar_tensor_tensor(
                    out_sb[:, it, :], t1[:], -1.0, out_ps[:], AL.add, AL.max
                )

            # 6-stage pipeline; emission order makes each engine's stream
            # oldest-dependency-first.
            for k in range(n_tiles + 6):
                if k < n_tiles:
                    phase_load(k)
                if 6 <= k:
                    phase_fin_pe(k - 6)
                if 4 <= k < n_tiles + 4:
                    phase_diag(k - 4)
                if 2 <= k < n_tiles + 2:
                    phase_dots(k - 2)
                if 3 <= k < n_tiles + 3:
                    phase_e(k - 3)
                if 5 <= k < n_tiles + 5:
                    phase_wsum(k - 5)
                    phase_copies(k - 5)
                if 6 <= k:
                    phase_elu(k - 6)
                    it_done = k - 6
                    if it_done % 4 == 3:
                        i0 = it_done - 3
                        nc.sync.dma_start(
                            out_d[:, i0 * D : (i0 + 4) * D],
                            out_sb[:, i0 : i0 + 4, :],
                        )

    nc.compile()
    return nc


_NC_CACHE = {}


def _get_nc(b_shard):
    if b_shard not in _NC_CACHE:
        _NC_CACHE[b_shard] = build_nc(b_shard)
    return _NC_CACHE[b_shard]


def _host_precompute(W_ally, W_opp, a_ally, a_opp, mask):
    v1a = W_ally @ a_ally[:D, 0]
    v2a = W_ally @ a_ally[D:, 0]
    v1o = W_opp @ a_opp[:D, 0]
    v2o = W_opp @ a_opp[D:, 0]
    vcat = np.ascontiguousarray(np.stack([v1a, v2a, v1o, v2o], axis=1).astype(BF16_NP))
    wcat = np.ascontiguousarray(np.concatenate([W_ally, W_opp], axis=1).astype(BF16_NP))
    b = mask.shape[0]
    mneg = np.zeros((b, NJ), np.float32)
    mneg[:, 1:21] = np.where(mask[:, 1 : 1 + NA], NEG_INF, 0.0)
    mneg[:, 22:42] = np.where(mask[:, 1 + NA :], NEG_INF, 0.0)
    return vcat, wcat, mneg


def kernel(h, W_ally, W_opp, a_ally, a_opp, mask, num_ally, num_opp):
    assert int(num_ally) == NA and int(num_opp) == NO
    h = np.asarray(h, dtype=np.float32)
    mask = np.asarray(mask)
    W_ally = np.asarray(W_ally, dtype=np.float32)
    W_opp = np.asarray(W_opp, dtype=np.float32)
    a_ally = np.asarray(a_ally, dtype=np.float32)
    a_opp = np.asarray(a_opp, dtype=np.float32)

    vcat, wcat, mneg = _host_precompute(W_ally, W_opp, a_ally, a_opp, mask)
    bfull = h.shape[0]
    h_bf3 = h.astype(BF16_NP)  # [B, NN, D]
    h_bf = np.ascontiguousarray(h_bf3.reshape(bfull, NN * D))
    # hT layout: per tile t of 128 rows, hT[t*128 + d, n*128 + b] = h[t*128 + b, n, d]
    n_tiles_full = bfull // P
    hT_bf = np.ascontiguousarray(
        h_bf3.reshape(n_tiles_full, P, NN, D).transpose(0, 3, 2, 1)
    ).reshape(bfull, NN * D)
    # mneg shuffled to [b_in_tile, tile, col] per core
    mneg_sh = np.ascontiguousarray(
        mneg.reshape(N_CORES, N_TILES, P, NJ).transpose(0, 2, 1, 3)
    ).reshape(N_CORES, P, N_TILES * NJ)

    nc = _get_nc(B_SHARD)
    in_maps = []
    for c in range(N_CORES):
        s = slice(c * B_SHARD, (c + 1) * B_SHARD)
        in_maps.append(
            {
                "h": h_bf[s],
                "hT": hT_bf[s],
                "mneg": mneg_sh[c],
                "vcat": vcat,
                "wcat": wcat,
            }
        )
    res = run_bass_kernel_spmd(nc, in_maps, core_ids=list(range(N_CORES)))
    global LAST_RESULTS
    LAST_RESULTS = res
    # out stored [b_in_tile, tile, col] -> [tile*128 + b, col]
    return np.concatenate(
        [
            np.ascontiguousarray(
                res.results[c]["out"].reshape(P, N_TILES, D).transpose(1, 0, 2)
            ).reshape(B_SHARD, D)
            for c in range(N_CORES)
        ],
        axis=0,
    )


LAST_RESULTS = None
